# revision 46
# baseline (speedup 1.0000x reference)
"""Trainium2 Bass kernel for nn_EquivariantLocalScoreMachine.

Math: for each query pixel q (B*H*W=4096, 27-dim circular 3x3 patch xq) over
dataset patches p (N*H*W=32768, zero-padded 3x3 patches):
    log_w[q,p] = -(||xq - mu*patch_p||^2) / (2 sigma^2)
               = b[q] + a*<xq, patch_p> + c[p],   a = mu/sigma^2
The per-q term b[q] multiplies every patch equally and cancels in the final
ratio, so it is dropped.  Output:
    out[q,ch] = (mu * wc[q,ch]/sum_w[q] - x[q,ch]) / sigma^2
with softmax-style weights over p.

Sharding: the patch axis is split across 8 cores (4096 patches each); each
core computes partial (sum_w, wc) for all queries under its own per-q shift,
and the host merges with an exact logsumexp rescale in fp64.

Per core:
  1. subset pass: bf16 matmul of g = a<xq,patch>+c over a stride-4 subset of
     local patches -> DVE reduce_max -> per-q shift s = g_sub + MARGIN.
     Since g <= g_sub + gap (gap small, empirically <80, overflow at 88+MARGIN)
     exp(g-s) never overflows, and the top weight >= e^-MARGIN never
     underflows.  The shift row (-s) is transposed into row 0 of the shared
     rhs via a small PE matmul against the identity and exported so the host
     knows the exact shift used.
  2. main pass: fp32 scores from ONE bf16 matmul per (p-chunk, q-tile) using
     a hi/lo split packed into the contraction dim (K=111: 4x27 cross terms
     xh*ph+xh*pl+xl*ph+xl*pl + c_hi + c_lo + shift row) -- same PE cost as
     K=32 since matmul time is set by the streamed free dim, and ~6x faster
     than the HW's fp32 LOW_HIGH mode.  ScalarE exp's PSUM->SBUF as bf16;
     a bf16 matmul against [pc_hi(3) | pc_lo(3) | 1] accumulates wc (split)
     and sum_w over all 32 p-chunks into an [8,512] PSUM accumulator.

Every TPB instruction in this walrus build may carry at most ONE sync wait:
tiny PE "fence" matmuls pre-absorb cross-engine semaphores on hot paths, and
a post-scheduling pass splits any remaining multi-wait instruction into
single-wait NoOps.
"""
import sys
import numpy as np

for _p in ("/opt/trn_rl_repo", "/opt/pypackages"):
    if _p not in sys.path:
        sys.path.append(_p)

import ml_dtypes

BF16 = ml_dtypes.bfloat16

B, C, H, W = 4, 3, 32, 32
N_IMG = 32
NQ = B * H * W            # 4096 queries
NP = N_IMG * H * W        # 32768 dataset patches
NCORES = 8
PLOC = NP // NCORES       # 4096 patches per core
NCHUNK = PLOC // 128      # 32 p-chunks per core
NQC = NQ // 128           # 32 q-chunks (subset pass)
NT = NQ // 512            # 8 q-tiles (main pass)
FD = 1024                 # A-tile free dim (2 chunks per exp call)
SUB_STRIDE = 16
NSUB = PLOC // SUB_STRIDE  # 256 subset patches per core (max gap 95 < 128)
MARGIN = 40.0
KA = 111                  # packed contraction: 4*27 + c_hi + c_lo + shift

_prog_cache = {}


def _build_program():
    if "nc" in _prog_cache:
        return _prog_cache["nc"]
    from contextlib import ExitStack
    import concourse.bass as bass
    import concourse.tile as tile
    from concourse import mybir

    f32 = mybir.dt.float32
    bf = mybir.dt.bfloat16
    nc = bass.Bass("TRN2", num_devices=NCORES, debug=False)
    patm_d = nc.dram_tensor("patm", [KA, PLOC], bf, kind="ExternalInput").ap()
    xa_d = nc.dram_tensor("xa", [KA, NQ], bf, kind="ExternalInput").ap()
    xs_d = nc.dram_tensor("xs", [32, NQ], bf, kind="ExternalInput").ap()
    subp_d = nc.dram_tensor("subp", [32, NSUB], bf, kind="ExternalInput").ap()
    pw_d = nc.dram_tensor("pw", [128, 256], bf, kind="ExternalInput").ap()
    ident_d = nc.dram_tensor("ident", [128, 128], bf, kind="ExternalInput").ap()
    out_d = nc.dram_tensor("out", [8, NQ], f32, kind="ExternalOutput").ap()
    srow_d = nc.dram_tensor("srow", [1, NQ], bf, kind="ExternalOutput").ap()

    with tile.TileContext(nc) as tc, ExitStack() as ctx:
        consts = ctx.enter_context(tc.tile_pool(name="consts", bufs=1))
        # PSUM (8 banks): psA 2x[128,1024]=4, ps_sub 1x[128,1024]=2,
        # rowp/wfence shared tag=1, acc 1x[8,512]=1
        ps_big = ctx.enter_context(tc.tile_pool(name="ps_big", bufs=2, space="PSUM"))
        ps_sub_pool = ctx.enter_context(
            tc.tile_pool(name="ps_sub_pool", bufs=1, space="PSUM"))
        ps_fence = ctx.enter_context(
            tc.tile_pool(name="ps_fence", bufs=1, space="PSUM"))
        ps_acc = ctx.enter_context(tc.tile_pool(name="ps_acc", bufs=2, space="PSUM"))
        wpool = ctx.enter_context(tc.tile_pool(name="wpool", bufs=3))
        spool = ctx.enter_context(tc.tile_pool(name="spool", bufs=2))
        dpool = ctx.enter_context(tc.tile_pool(name="dpool", bufs=2))

        # All sync DMAs serialize on one HWDGE queue (~100GB/s), so order by
        # first use. xa row 0 is memset on-device (the shift writes replace
        # it) so the shift path never waits on the 0.9MB bulk transfer; patm
        # is split so the first p-chunks arrive before the main loop needs
        # them.
        s_subp = consts.tile([32, NSUB], bf)
        nc.sync.dma_start(out=s_subp, in_=subp_d)
        s_xs = consts.tile([32, NQ], bf)
        nc.sync.dma_start(out=s_xs, in_=xs_d)
        s_id = consts.tile([128, 128], bf)
        nc.sync.dma_start(out=s_id, in_=ident_d)
        s_xa = consts.tile([KA, NQ], bf)
        nc.vector.memset(s_xa[0:1, :], 0.0)
        # q-tile 0's rhs slice and the first p-chunks first, so the main loop
        # starts as soon as the serial DMA queue delivers them
        nc.sync.dma_start(out=s_xa[1:KA, 0:512], in_=xa_d[1:KA, 0:512])
        s_patm = consts.tile([KA, PLOC], bf)
        nc.sync.dma_start(out=s_patm[:, 0:PLOC // 2], in_=patm_d[:, 0:PLOC // 2])
        nc.sync.dma_start(out=s_xa[1:KA, 512:], in_=xa_d[1:KA, 512:])
        s_pw = consts.tile([128, 256], bf)
        nc.sync.dma_start(out=s_pw, in_=pw_d)
        nc.sync.dma_start(out=s_patm[:, PLOC // 2:], in_=patm_d[:, PLOC // 2:])

        # wfence: long-lived scratch bank. Fence matmuls write junk columns;
        # [128:256] holds each q-chunk's transposed shift row briefly (read
        # by the TS copy before the next chunk's transpose overwrites it).
        wfence = ps_fence.tile([1, 512], f32, name="wfence")

        def warm_fence(wi, warm):
            nc.tensor.matmul(wfence[0:1, wi:wi + 1], warm[0:32, 0:1],
                             warm[0:32, 0:1], start=True, stop=True)

        # Subset-max work for one q-chunk: bf16 scores over the patch subset,
        # DVE reduce_max, then -(g_sub+MARGIN) transposed into xa row 0.
        gsubs = {}

        def subset_chunk(qc, pool):
            if qc >= 1:
                # absorb the DVE tick of reduce(qc-1), whose PSUM slot the
                # matmuls below (or soon after) reuse
                nc.tensor.matmul(wfence[0:1, 8 + (qc % 2):9 + (qc % 2)],
                                 gsubs[qc - 1], s_id[0:128, 0:1],
                                 start=True, stop=True)
            ps_sub = pool.tile([128, NSUB], f32,
                               tag="big" if pool is ps_big else "sub",
                               name="ps_sub")
            for h in range((NSUB + 511) // 512):
                n0, n1 = h * 512, min((h + 1) * 512, NSUB)
                nc.tensor.matmul(
                    ps_sub[:, n0:n1],
                    s_xs[:, qc * 128:(qc + 1) * 128],
                    s_subp[:, n0:n1],
                    start=True, stop=True,
                )
            # bf16 shift (error < +-2, absorbed by MARGIN) keeps the transpose
            # matmul in single-pass bf16 instead of fp32 LOW_HIGH
            gsub = spool.tile([128, 1], bf, name="gsub")
            gsubs[qc] = gsub
            nc.vector.reduce_max(out=gsub, in_=ps_sub, axis=mybir.AxisListType.X)
            rowp = wfence[0:1, 128:256]
            nc.tensor.matmul(rowp, gsub, s_id, start=True, stop=True)
            # xa row 0 <- -(g_sub + MARGIN), converted to bf16 on write
            nc.vector.tensor_scalar(
                s_xa[0:1, qc * 128:(qc + 1) * 128], rowp,
                MARGIN, -1.0, mybir.AluOpType.add, mybir.AluOpType.mult,
            )

        # Main work for one q-tile: shifted log-weights -> exp(bf16) ->
        # [wc_hi|wc_lo|sum_w] accumulated over all 32 p-chunks. The next
        # tile's subset chunks are woven between pr-groups so their PE/DVE
        # ping-pong overlaps this tile's exp stream instead of stalling it.
        def main_tile(t, weave):
            # absorb the DVE ticks of this q-tile's row-0 shift writes
            nc.tensor.matmul(wfence[0:1, 0:512],
                             s_xa[0:1, t * 512:t * 512 + 1],
                             s_xa[0:1, t * 512:(t + 1) * 512],
                             start=True, stop=True)
            accB = ps_acc.tile([8, 512], f32, tag="acc", name="accB")
            for pr in range(NCHUNK // 2):
                psA = ps_big.tile([128, FD], f32, tag="big", name="psA")
                for i in range(2):
                    ch = pr * 2 + i
                    nc.tensor.matmul(
                        psA[:, i * 512:(i + 1) * 512],
                        s_patm[:, ch * 128:(ch + 1) * 128],
                        s_xa[:, t * 512:(t + 1) * 512],
                        start=True, stop=True,
                    )
                wgt = wpool.tile([128, FD], bf, name="wgt")
                nc.scalar.activation(wgt, psA, mybir.ActivationFunctionType.Exp)
                for i in range(2):
                    ch = pr * 2 + i
                    nc.tensor.matmul(
                        accB,
                        s_pw[:, ch * 8:(ch + 1) * 8],
                        wgt[:, i * 512:(i + 1) * 512],
                        start=(ch == 0), stop=(ch == NCHUNK - 1),
                    )
                if weave and pr in (2, 6, 10, 14):
                    subset_chunk(weave.pop(0), ps_sub_pool)
            dr = dpool.tile([8, 512], f32, name="dr")
            nc.vector.tensor_copy(dr, accB)
            nc.sync.dma_start(out=out_d[:, t * 512:(t + 1) * 512], in_=dr)

        for wi, warm in enumerate((s_subp, s_xs, s_id)):
            warm_fence(wi, warm)
        for qc in range(4):
            subset_chunk(qc, ps_big)
        for wi, warm in enumerate((s_xa, s_patm, s_pw)):
            warm_fence(3 + wi, warm)
        for t in range(NT):
            weave = list(range(4 * t + 4, 4 * t + 8)) if t < NT - 1 else []
            main_tile(t, weave)

        nc.sync.dma_start(out=srow_d, in_=s_xa[0:1, :])

    # This walrus build permits exactly ONE sync wait per instruction
    # (setupSyncWait raises "Too many sync wait commands" at 2). Tile emits
    # several on converging dependencies; move extras onto preceding
    # same-engine NoOps, each carrying a single wait.
    for blk in nc.m.functions[0].blocks:
        out, changed = [], False
        for ins in blk.instructions:
            si = ins.sync_info
            w = list(si.on_wait) if si is not None and si.on_wait else []
            if len(w) > 1:
                for k, extra in enumerate(w[:-1]):
                    out.append(mybir.InstNoOp(
                        name=f"{ins.name}_sw{k}", engine=ins.engine,
                        sync_info=mybir.SyncInfo(on_wait=[extra], on_update=[]),
                        bass_nofuse=True))
                ins.sync_info = mybir.SyncInfo(
                    on_wait=[w[-1]], on_update=list(si.on_update))
                changed = True
            out.append(ins)
        if changed:
            blk.instructions = out

    _prog_cache["nc"] = nc
    return nc


def _im2col(arr, wrap):
    # [M,C,H,W] -> [M*H*W, C*9]; row m*H*W + h*W + w; col c*9 + di*3 + dj
    if wrap:
        ap = np.pad(arr, ((0, 0), (0, 0), (1, 1), (1, 1)), mode="wrap")
    else:
        ap = np.pad(arr, ((0, 0), (0, 0), (1, 1), (1, 1)))
    sl = [ap[:, :, di:di + H, dj:dj + W] for di in range(3) for dj in range(3)]
    st = np.stack(sl, axis=2)  # [M,C,9,H,W]
    return st.transpose(0, 3, 4, 1, 2).reshape(arr.shape[0] * H * W, C * 9)


def _split(v):
    # fp32 -> (hi, lo) bf16 pair with v ~= hi + lo to ~2^-16 relative
    h = v.astype(BF16)
    l = (v - h.astype(np.float32)).astype(BF16)
    return h, l


def _run(inputs, trace=False):
    from concourse.bass_utils import run_bass_kernel_spmd

    x = np.ascontiguousarray(np.asarray(inputs["x"], dtype=np.float32))
    images = np.ascontiguousarray(np.asarray(inputs["images"], dtype=np.float32))
    t = int(np.asarray(inputs["t"]))
    mu = float(np.asarray(inputs["mu_sched"])[t])
    sigma = float(np.asarray(inputs["sigma_sched"])[t])
    a = mu / (sigma * sigma)
    inv2s2 = 1.0 / (2.0 * sigma * sigma)

    xq = _im2col(x, wrap=True)                  # [NQ, 27]
    patches = _im2col(images, wrap=False)       # [NP, 27]
    c_all = ((-mu * mu * inv2s2) * np.sum(patches * patches, axis=1)).astype(np.float32)
    pcent = patches.reshape(NP, C, 9)[:, :, 4]  # [NP, 3]

    xh, xl = _split((a * xq).astype(np.float32))   # [NQ, 27] bf16 each

    # x-side packed rhs: row0 = -s (device), then 4x27 split rows, c rows
    xa = np.zeros((KA, NQ), BF16)
    xa[1:28] = xh.T
    xa[28:55] = xh.T
    xa[55:82] = xl.T
    xa[82:109] = xl.T
    xa[109] = BF16(1.0)
    xa[110] = BF16(1.0)

    xs = np.zeros((32, NQ), BF16)               # subset-pass lhsT (hi only)
    xs[0:27] = xh.T
    xs[27] = BF16(1.0)

    ident = np.eye(128, dtype=np.float32).astype(BF16)

    in_maps = []
    for cc in range(NCORES):
        lo = cc * PLOC
        ph, pl = _split(patches[lo:lo + PLOC])
        ch, cl = _split(c_all[lo:lo + PLOC])
        patm = np.zeros((KA, PLOC), BF16)
        patm[0] = BF16(1.0)
        patm[1:28] = ph.T
        patm[28:55] = pl.T
        patm[55:82] = ph.T
        patm[82:109] = pl.T
        patm[109] = ch
        patm[110] = cl
        subp = np.zeros((32, NSUB), BF16)
        subp[0:27] = ph.T[:, ::SUB_STRIDE]
        subp[27] = ch[::SUB_STRIDE]
        pwh, pwl = _split(pcent[lo:lo + PLOC])
        pw = np.zeros((128, 256), BF16)
        for chnk in range(NCHUNK):
            pw[:, chnk * 8:chnk * 8 + 3] = pwh[chnk * 128:(chnk + 1) * 128]
            pw[:, chnk * 8 + 3:chnk * 8 + 6] = pwl[chnk * 128:(chnk + 1) * 128]
            pw[:, chnk * 8 + 6] = BF16(1.0)
        in_maps.append({
            "patm": patm, "xa": xa, "xs": xs, "subp": subp, "pw": pw,
            "ident": ident,
        })

    nc = _build_program()
    res = run_bass_kernel_spmd(nc, in_maps, core_ids=list(range(NCORES)),
                               trace=trace)

    # host merge: partials are scaled by e^{-s_c}; rescale to common max
    s = np.stack([-r["srow"][0].astype(np.float32) for r in res.results])
    part = np.stack([r["out"] for r in res.results])          # [8, 8, NQ]
    S = s.max(axis=0)
    fac = np.exp((s - S[None, :]).astype(np.float64))          # [8, NQ] <= 1
    sum_w = (part[:, 6].astype(np.float64) * fac).sum(axis=0)              # [NQ]
    wc = ((part[:, 0:3] + part[:, 3:6]).astype(np.float64)
          * fac[:, None, :]).sum(axis=0)                       # [3, NQ]

    xcT = x.reshape(B, C, H * W).transpose(1, 0, 2).reshape(C, NQ)
    out_q = (mu * wc / sum_w[None, :] - xcT) / (sigma * sigma)  # [3, NQ]
    out = out_q.reshape(C, B, H, W).transpose(1, 0, 2, 3).astype(np.float32)
    return out, res


def kernel(**inputs) -> np.ndarray:
    out, _ = _run(inputs, trace=False)
    return out


# revision 51
# speedup vs baseline: 1.0772x; 1.0772x over previous
"""Trainium2 Bass kernel for nn_EquivariantLocalScoreMachine.

Math: for each query pixel q (B*H*W=4096, 27-dim circular 3x3 patch xq) over
dataset patches p (N*H*W=32768, zero-padded 3x3 patches):
    log_w[q,p] = -(||xq - mu*patch_p||^2) / (2 sigma^2)
               = b[q] + a*<xq, patch_p> + c[p],   a = mu/sigma^2
The per-q term b[q] multiplies every patch equally and cancels in the final
ratio, so it is dropped.  Output:
    out[q,ch] = (mu * wc[q,ch]/sum_w[q] - x[q,ch]) / sigma^2
with softmax-style weights over p.

Sharding: the patch axis is split across 8 cores (4096 patches each); each
core computes partial (sum_w, wc) for all queries under its own per-q shift,
and the host merges with an exact logsumexp rescale in fp64.

Per core:
  1. subset pass: bf16 matmul of g = a<xq,patch>+c over a stride-4 subset of
     local patches -> DVE reduce_max -> per-q shift s = g_sub + MARGIN.
     Since g <= g_sub + gap (gap small, empirically <80, overflow at 88+MARGIN)
     exp(g-s) never overflows, and the top weight >= e^-MARGIN never
     underflows.  The shift row (-s) is transposed into row 0 of the shared
     rhs via a small PE matmul against the identity and exported so the host
     knows the exact shift used.
  2. main pass: fp32 scores from ONE bf16 matmul per (p-chunk, q-tile) using
     a hi/lo split packed into the contraction dim (K=111: 4x27 cross terms
     xh*ph+xh*pl+xl*ph+xl*pl + c_hi + c_lo + shift row) -- same PE cost as
     K=32 since matmul time is set by the streamed free dim, and ~6x faster
     than the HW's fp32 LOW_HIGH mode.  ScalarE exp's PSUM->SBUF as bf16;
     a bf16 matmul against [pc_hi(3) | pc_lo(3) | 1] accumulates wc (split)
     and sum_w over all 32 p-chunks into an [8,512] PSUM accumulator.

Every TPB instruction in this walrus build may carry at most ONE sync wait:
tiny PE "fence" matmuls pre-absorb cross-engine semaphores on hot paths, and
a post-scheduling pass splits any remaining multi-wait instruction into
single-wait NoOps.
"""
import sys
import numpy as np

for _p in ("/opt/trn_rl_repo", "/opt/pypackages"):
    if _p not in sys.path:
        sys.path.append(_p)

import ml_dtypes

BF16 = ml_dtypes.bfloat16

B, C, H, W = 4, 3, 32, 32
N_IMG = 32
NQ = B * H * W            # 4096 queries
NP = N_IMG * H * W        # 32768 dataset patches
NCORES = 8
PLOC = NP // NCORES       # 4096 patches per core
NCHUNK = PLOC // 128      # 32 p-chunks per core
NQC = NQ // 128           # 32 q-chunks (subset pass)
NT = NQ // 512            # 8 q-tiles (main pass)
FD = 1024                 # A-tile free dim (2 chunks per exp call)
SUB_STRIDE = 16
NSUB = PLOC // SUB_STRIDE  # 256 subset patches per core (max gap 95 < 128)
MARGIN = 40.0
KA = 111                  # packed contraction: 4*27 + c_hi + c_lo + shift

_prog_cache = {}


def _build_program():
    if "nc" in _prog_cache:
        return _prog_cache["nc"]
    from contextlib import ExitStack
    import concourse.bass as bass
    import concourse.tile as tile
    from concourse import mybir

    f32 = mybir.dt.float32
    bf = mybir.dt.bfloat16
    nc = bass.Bass("TRN2", num_devices=NCORES, debug=False)
    patm_d = nc.dram_tensor("patm", [KA, PLOC], bf, kind="ExternalInput").ap()
    xa_d = nc.dram_tensor("xa", [KA, NQ], bf, kind="ExternalInput").ap()
    xs_d = nc.dram_tensor("xs", [32, NQ], bf, kind="ExternalInput").ap()
    subp_d = nc.dram_tensor("subp", [32, NSUB], bf, kind="ExternalInput").ap()
    pw_d = nc.dram_tensor("pw", [128, 256], bf, kind="ExternalInput").ap()
    ident_d = nc.dram_tensor("ident", [128, 128], bf, kind="ExternalInput").ap()
    out_d = nc.dram_tensor("out", [8, NQ], f32, kind="ExternalOutput").ap()
    srow_d = nc.dram_tensor("srow", [1, NQ], bf, kind="ExternalOutput").ap()

    with tile.TileContext(nc) as tc, ExitStack() as ctx:
        consts = ctx.enter_context(tc.tile_pool(name="consts", bufs=1))
        # PSUM (8 banks): psA 3x[128,1024]=6, shared scratch bank 1 (subset
        # scores + fence junk + transposed shift row), acc 1x[8,512]=1
        ps_big = ctx.enter_context(tc.tile_pool(name="ps_big", bufs=3, space="PSUM"))
        ps_fence = ctx.enter_context(
            tc.tile_pool(name="ps_fence", bufs=1, space="PSUM"))
        ps_acc = ctx.enter_context(tc.tile_pool(name="ps_acc", bufs=1, space="PSUM"))
        wpool = ctx.enter_context(tc.tile_pool(name="wpool", bufs=3))
        spool = ctx.enter_context(tc.tile_pool(name="spool", bufs=2))
        dpool = ctx.enter_context(tc.tile_pool(name="dpool", bufs=2))

        # All sync DMAs serialize on one HWDGE queue (~100GB/s), so order by
        # first use. xa row 0 is memset on-device (the shift writes replace
        # it) so the shift path never waits on the 0.9MB bulk transfer; patm
        # is split so the first p-chunks arrive before the main loop needs
        # them.
        s_subp = consts.tile([32, NSUB], bf)
        nc.sync.dma_start(out=s_subp, in_=subp_d)
        s_xs = consts.tile([32, NQ], bf)
        nc.sync.dma_start(out=s_xs, in_=xs_d)
        s_id = consts.tile([128, 128], bf)
        nc.sync.dma_start(out=s_id, in_=ident_d)
        s_xa = consts.tile([KA, NQ], bf)
        nc.vector.memset(s_xa[0:1, :], 0.0)
        # q-tile 0's rhs slice and the first p-chunks first, so the main loop
        # starts as soon as the serial DMA queue delivers them
        nc.sync.dma_start(out=s_xa[1:KA, 0:512], in_=xa_d[1:KA, 0:512])
        s_patm = consts.tile([KA, PLOC], bf)
        nc.sync.dma_start(out=s_patm[:, 0:PLOC // 2], in_=patm_d[:, 0:PLOC // 2])
        nc.sync.dma_start(out=s_xa[1:KA, 512:], in_=xa_d[1:KA, 512:])
        s_pw = consts.tile([128, 256], bf)
        nc.sync.dma_start(out=s_pw, in_=pw_d)
        nc.sync.dma_start(out=s_patm[:, PLOC // 2:], in_=patm_d[:, PLOC // 2:])

        # wfence: long-lived scratch bank. Fence matmuls write junk columns;
        # [128:256] holds each q-chunk's transposed shift row briefly (read
        # by the TS copy before the next chunk's transpose overwrites it).
        # One persistent scratch bank: fence columns [0:16), the transposed
        # shift row at [128:256), and the subset score region at [256:256+NSUB)
        wfence = ps_fence.tile([128, 512], f32, name="wfence")

        def warm_fence(wi, warm):
            nc.tensor.matmul(wfence[0:1, wi:wi + 1], warm[0:32, 0:1],
                             warm[0:32, 0:1], start=True, stop=True)

        # Subset-max work for one q-chunk: bf16 scores over the patch subset,
        # DVE reduce_max, then -(g_sub+MARGIN) transposed into xa row 0.
        gsubs = {}

        def subset_chunk(qc):
            if qc >= 1:
                # absorb the DVE tick of reduce(qc-1), whose scratch region
                # the matmuls below overwrite
                nc.tensor.matmul(wfence[0:1, 8 + (qc % 2):9 + (qc % 2)],
                                 gsubs[qc - 1], s_id[0:128, 0:1],
                                 start=True, stop=True)
            ps_sub = wfence[:, 256:256 + NSUB]
            for h in range((NSUB + 511) // 512):
                n0, n1 = h * 512, min((h + 1) * 512, NSUB)
                nc.tensor.matmul(
                    ps_sub[:, n0:n1],
                    s_xs[:, qc * 128:(qc + 1) * 128],
                    s_subp[:, n0:n1],
                    start=True, stop=True,
                )
            # bf16 shift (error < +-2, absorbed by MARGIN) keeps the transpose
            # matmul in single-pass bf16 instead of fp32 LOW_HIGH
            gsub = spool.tile([128, 1], bf, name="gsub")
            gsubs[qc] = gsub
            nc.vector.reduce_max(out=gsub, in_=ps_sub, axis=mybir.AxisListType.X)
            rowp = wfence[0:1, 128:256]
            nc.tensor.matmul(rowp, gsub, s_id, start=True, stop=True)
            # xa row 0 <- -(g_sub + MARGIN), converted to bf16 on write
            nc.vector.tensor_scalar(
                s_xa[0:1, qc * 128:(qc + 1) * 128], rowp,
                MARGIN, -1.0, mybir.AluOpType.add, mybir.AluOpType.mult,
            )

        # Main work for one q-tile: shifted log-weights -> exp(bf16) ->
        # [wc_hi|wc_lo|sum_w] accumulated over all 32 p-chunks. The next
        # tile's subset chunks are woven between pr-groups so their PE/DVE
        # ping-pong overlaps this tile's exp stream instead of stalling it.
        def main_tile(t, weave):
            # absorb the DVE ticks of this q-tile's row-0 shift writes
            nc.tensor.matmul(wfence[0:1, 0:512],
                             s_xa[0:1, t * 512:t * 512 + 1],
                             s_xa[0:1, t * 512:(t + 1) * 512],
                             start=True, stop=True)
            accB = ps_acc.tile([8, 512], f32, tag="acc", name="accB")
            for pr in range(NCHUNK // 2):
                psA = ps_big.tile([128, FD], f32, tag="big", name="psA")
                for i in range(2):
                    ch = pr * 2 + i
                    nc.tensor.matmul(
                        psA[:, i * 512:(i + 1) * 512],
                        s_patm[:, ch * 128:(ch + 1) * 128],
                        s_xa[:, t * 512:(t + 1) * 512],
                        start=True, stop=True,
                    )
                wgt = wpool.tile([128, FD], bf, name="wgt")
                nc.scalar.activation(wgt, psA, mybir.ActivationFunctionType.Exp)
                for i in range(2):
                    ch = pr * 2 + i
                    nc.tensor.matmul(
                        accB,
                        s_pw[:, ch * 8:(ch + 1) * 8],
                        wgt[:, i * 512:(i + 1) * 512],
                        start=(ch == 0), stop=(ch == NCHUNK - 1),
                    )
                if weave and pr in (2, 6, 10, 14):
                    subset_chunk(weave.pop(0))
            dr = dpool.tile([8, 512], f32, name="dr")
            nc.vector.tensor_copy(dr, accB)
            nc.sync.dma_start(out=out_d[:, t * 512:(t + 1) * 512], in_=dr)

        for wi, warm in enumerate((s_subp, s_xs, s_id)):
            warm_fence(wi, warm)
        for qc in range(4):
            subset_chunk(qc)
        for wi, warm in enumerate((s_xa, s_patm, s_pw)):
            warm_fence(3 + wi, warm)
        for t in range(NT):
            weave = list(range(4 * t + 4, 4 * t + 8)) if t < NT - 1 else []
            main_tile(t, weave)

        nc.sync.dma_start(out=srow_d, in_=s_xa[0:1, :])

    # This walrus build permits exactly ONE sync wait per instruction
    # (setupSyncWait raises "Too many sync wait commands" at 2). Tile emits
    # several on converging dependencies; move extras onto preceding
    # same-engine NoOps, each carrying a single wait.
    for blk in nc.m.functions[0].blocks:
        out, changed = [], False
        for ins in blk.instructions:
            si = ins.sync_info
            w = list(si.on_wait) if si is not None and si.on_wait else []
            if len(w) > 1:
                for k, extra in enumerate(w[:-1]):
                    out.append(mybir.InstNoOp(
                        name=f"{ins.name}_sw{k}", engine=ins.engine,
                        sync_info=mybir.SyncInfo(on_wait=[extra], on_update=[]),
                        bass_nofuse=True))
                ins.sync_info = mybir.SyncInfo(
                    on_wait=[w[-1]], on_update=list(si.on_update))
                changed = True
            out.append(ins)
        if changed:
            blk.instructions = out

    _prog_cache["nc"] = nc
    return nc


def _im2col(arr, wrap):
    # [M,C,H,W] -> [M*H*W, C*9]; row m*H*W + h*W + w; col c*9 + di*3 + dj
    if wrap:
        ap = np.pad(arr, ((0, 0), (0, 0), (1, 1), (1, 1)), mode="wrap")
    else:
        ap = np.pad(arr, ((0, 0), (0, 0), (1, 1), (1, 1)))
    sl = [ap[:, :, di:di + H, dj:dj + W] for di in range(3) for dj in range(3)]
    st = np.stack(sl, axis=2)  # [M,C,9,H,W]
    return st.transpose(0, 3, 4, 1, 2).reshape(arr.shape[0] * H * W, C * 9)


def _split(v):
    # fp32 -> (hi, lo) bf16 pair with v ~= hi + lo to ~2^-16 relative
    h = v.astype(BF16)
    l = (v - h.astype(np.float32)).astype(BF16)
    return h, l


def _run(inputs, trace=False):
    from concourse.bass_utils import run_bass_kernel_spmd

    x = np.ascontiguousarray(np.asarray(inputs["x"], dtype=np.float32))
    images = np.ascontiguousarray(np.asarray(inputs["images"], dtype=np.float32))
    t = int(np.asarray(inputs["t"]))
    mu = float(np.asarray(inputs["mu_sched"])[t])
    sigma = float(np.asarray(inputs["sigma_sched"])[t])
    a = mu / (sigma * sigma)
    inv2s2 = 1.0 / (2.0 * sigma * sigma)

    xq = _im2col(x, wrap=True)                  # [NQ, 27]
    patches = _im2col(images, wrap=False)       # [NP, 27]
    c_all = ((-mu * mu * inv2s2) * np.sum(patches * patches, axis=1)).astype(np.float32)
    pcent = patches.reshape(NP, C, 9)[:, :, 4]  # [NP, 3]

    xh, xl = _split((a * xq).astype(np.float32))   # [NQ, 27] bf16 each

    # x-side packed rhs: row0 = -s (device), then 4x27 split rows, c rows
    xa = np.zeros((KA, NQ), BF16)
    xa[1:28] = xh.T
    xa[28:55] = xh.T
    xa[55:82] = xl.T
    xa[82:109] = xl.T
    xa[109] = BF16(1.0)
    xa[110] = BF16(1.0)

    xs = np.zeros((32, NQ), BF16)               # subset-pass lhsT (hi only)
    xs[0:27] = xh.T
    xs[27] = BF16(1.0)

    ident = np.eye(128, dtype=np.float32).astype(BF16)

    in_maps = []
    for cc in range(NCORES):
        lo = cc * PLOC
        ph, pl = _split(patches[lo:lo + PLOC])
        ch, cl = _split(c_all[lo:lo + PLOC])
        patm = np.zeros((KA, PLOC), BF16)
        patm[0] = BF16(1.0)
        patm[1:28] = ph.T
        patm[28:55] = pl.T
        patm[55:82] = ph.T
        patm[82:109] = pl.T
        patm[109] = ch
        patm[110] = cl
        subp = np.zeros((32, NSUB), BF16)
        subp[0:27] = ph.T[:, ::SUB_STRIDE]
        subp[27] = ch[::SUB_STRIDE]
        pwh, pwl = _split(pcent[lo:lo + PLOC])
        pw = np.zeros((128, 256), BF16)
        for chnk in range(NCHUNK):
            pw[:, chnk * 8:chnk * 8 + 3] = pwh[chnk * 128:(chnk + 1) * 128]
            pw[:, chnk * 8 + 3:chnk * 8 + 6] = pwl[chnk * 128:(chnk + 1) * 128]
            pw[:, chnk * 8 + 6] = BF16(1.0)
        in_maps.append({
            "patm": patm, "xa": xa, "xs": xs, "subp": subp, "pw": pw,
            "ident": ident,
        })

    nc = _build_program()
    res = run_bass_kernel_spmd(nc, in_maps, core_ids=list(range(NCORES)),
                               trace=trace)

    # host merge: partials are scaled by e^{-s_c}; rescale to common max
    s = np.stack([-r["srow"][0].astype(np.float32) for r in res.results])
    part = np.stack([r["out"] for r in res.results])          # [8, 8, NQ]
    S = s.max(axis=0)
    fac = np.exp((s - S[None, :]).astype(np.float64))          # [8, NQ] <= 1
    sum_w = (part[:, 6].astype(np.float64) * fac).sum(axis=0)              # [NQ]
    wc = ((part[:, 0:3] + part[:, 3:6]).astype(np.float64)
          * fac[:, None, :]).sum(axis=0)                       # [3, NQ]

    xcT = x.reshape(B, C, H * W).transpose(1, 0, 2).reshape(C, NQ)
    out_q = (mu * wc / sum_w[None, :] - xcT) / (sigma * sigma)  # [3, NQ]
    out = out_q.reshape(C, B, H, W).transpose(1, 0, 2, 3).astype(np.float32)
    return out, res


def kernel(**inputs) -> np.ndarray:
    out, _ = _run(inputs, trace=False)
    return out
